# revision 1
# baseline (speedup 1.0000x reference)
"""Proven-working v1 (998us, rel err 3.5e-4): natural-layout scores,
ACT accum_out row sums, DMA-transpose of probabilities. Kept as fallback."""

import numpy as np
import ml_dtypes

from concourse import bacc, bass, tile, mybir
from concourse.bass_utils import run_bass_kernel_spmd

B, L, D = 16, 1024, 512
A = D
NCORES = 8
BLOC = B // NCORES
P = 128
DC = D // P
AC = A // P
LT = L // P
KC = L // P
NH = 512
SCALE = float(1.0 / np.sqrt(np.float32(D)))

F32 = mybir.dt.float32
BF16 = mybir.dt.bfloat16
EXP = mybir.ActivationFunctionType.Exp
COPY = mybir.ActivationFunctionType.Copy

W_NAMES = [f"{blk}_{w}" for blk in ("ta", "va", "tv")
           for w in ("kx", "qx", "vx", "ky", "qy", "vy")] + [
    "tav_k", "tav_q", "tav_v"]


def _build():
    nc = bacc.Bacc("TRN2", target_bir_lowering=False, debug=False,
                   num_devices=NCORES)

    mt_txt = nc.dram_tensor("mt_txt", (BLOC, D, L), BF16, kind="ExternalInput").ap()
    mt_au = nc.dram_tensor("mt_au", (BLOC, D, L), BF16, kind="ExternalInput").ap()
    mt_vi = nc.dram_tensor("mt_vi", (BLOC, D, L), BF16, kind="ExternalInput").ap()
    res = nc.dram_tensor("res", (3, BLOC, L, D), F32, kind="ExternalInput").ap()
    wt = nc.dram_tensor("wt", (21, D, A), BF16, kind="ExternalInput").ap()
    out = nc.dram_tensor("out", (BLOC, L, 4 * A), F32, kind="ExternalOutput").ap()

    with tile.TileContext(nc) as tc:
        _body(nc, tc, mt_txt, mt_au, mt_vi, res, wt, out)

    nc.compile()
    return nc


def _body(nc, tc, mt_txt, mt_au, mt_vi, res, wt, out):
    mt_dram = {"txt": mt_txt, "au": mt_au, "vi": mt_vi}

    with (
        tc.tile_pool(name="persist", bufs=1) as persist,
        tc.tile_pool(name="wpool", bufs=1) as wpool,
        tc.tile_pool(name="mpool", bufs=1) as mpool,
        tc.tile_pool(name="proj", bufs=1) as projp,
        tc.tile_pool(name="attn", bufs=2) as attnp,
        tc.tile_pool(name="small", bufs=3) as smallp,
        tc.tile_pool(name="ps_big", bufs=2, space=bass.MemorySpace.PSUM) as psb,
        tc.tile_pool(name="ps_small", bufs=4, space=bass.MemorySpace.PSUM) as pss,
    ):
        avT = [persist.tile([P, AC, L], BF16, tag=f"avT{b}", name=f"avT{b}")
               for b in range(BLOC)]

        def load_w(j):
            t = wpool.tile([P, DC, A], BF16, tag=f"w{j % 6}", name=f"w{j}")
            nc.sync.dma_start(out=t[:, :, :],
                              in_=wt[j].rearrange("(dc p) a -> p dc a", p=P))
            return t

        def load_mt(name, b, slot):
            t = mpool.tile([P, DC, L], BF16, tag=f"mT{slot}_{b}", name=f"mT_{name}{b}")
            nc.sync.dma_start(out=t[:, :, :],
                              in_=mt_dram[name][b].rearrange("(dc p) l -> p dc l", p=P))
            return t

        def proj_T(wtile, mtile, tag):
            o = projp.tile([P, AC, L], BF16, tag=tag, name=tag)
            for ac in range(AC):
                for h in range(L // NH):
                    ps = pss.tile([P, NH], F32, tag="ps_s", name="ps_pt")
                    for dc in range(DC):
                        nc.tensor.matmul(ps[:, :],
                                         wtile[:, dc, ac * P:(ac + 1) * P],
                                         mtile[:, dc, h * NH:(h + 1) * NH],
                                         start=(dc == 0), stop=(dc == DC - 1))
                    nc.vector.tensor_copy(o[:, ac, h * NH:(h + 1) * NH], ps[:, :])
            return o

        def proj_N(wtile, mtile, tag):
            o = projp.tile([P, KC, A], BF16, tag=tag, name=tag)
            for lt in range(LT):
                ps = pss.tile([P, NH], F32, tag="ps_s", name="ps_pn")
                for dc in range(DC):
                    nc.tensor.matmul(ps[:, :],
                                     mtile[:, dc, lt * P:(lt + 1) * P],
                                     wtile[:, dc, :],
                                     start=(dc == 0), stop=(dc == DC - 1))
                nc.vector.tensor_copy(o[:, lt, :], ps[:, :])
            return o

        def attention(qT, kT, v, writer):
            probsT = attnp.tile([P, KC, L], BF16, tag="probsT", name="probsT")
            sums = smallp.tile([P, LT], F32, tag="sums", name="sums")
            recip = smallp.tile([P, LT], F32, tag="recip", name="recip")
            for qt in range(LT):
                ps = psb.tile([P, L], F32, tag="scores", name="scores")
                for kh in range(L // NH):
                    for ac in range(AC):
                        nc.tensor.matmul(ps[:, kh * NH:(kh + 1) * NH],
                                         qT[:, ac, qt * P:(qt + 1) * P],
                                         kT[:, ac, kh * NH:(kh + 1) * NH],
                                         start=(ac == 0), stop=(ac == AC - 1))
                probs = attnp.tile([P, L], BF16, tag="probs", name="probs")
                nc.scalar.activation(probs[:, :], ps[:, :], EXP, scale=SCALE,
                                     accum_out=sums[:, qt:qt + 1])
                nc.scalar.dma_start_transpose(
                    out=probsT[:, :, qt * P:(qt + 1) * P], in_=probs[:, :])
                nc.vector.reciprocal(recip[:, qt:qt + 1], sums[:, qt:qt + 1])
            for qt in range(LT):
                po = pss.tile([P, A], F32, tag="ps_s", name="ps_pv")
                for kc in range(KC):
                    nc.tensor.matmul(po[:, :],
                                     probsT[:, kc, qt * P:(qt + 1) * P],
                                     v[:, kc, :],
                                     start=(kc == 0), stop=(kc == KC - 1))
                writer(qt, po, recip[:, qt:qt + 1])

        blocks = [(0, "txt", "au", 0), (1, "vi", "au", 2), (2, "txt", "vi", 1)]
        for blk, n1, n2, col in blocks:
            w = [load_w(blk * 6 + j) for j in range(6)]
            for b in range(BLOC):
                m1T = load_mt(n1, b, 1)
                m2T = load_mt(n2, b, 2)
                k1T = proj_T(w[0], m1T, "k1T")
                q2T = proj_T(w[4], m2T, "q2T")
                v1 = proj_N(w[2], m1T, "v1")
                k2T = proj_T(w[3], m2T, "k2T")
                q1T = proj_T(w[1], m1T, "q1T")
                v2 = proj_N(w[5], m2T, "v2")

                o1n = projp.tile([P, LT, A], BF16, tag="o1n", name="o1n")

                def writer1(qt, po, rc):
                    nc.scalar.activation(o1n[:, qt, :], po[:, :], COPY, scale=rc)

                def writer2(qt, po, rc, blk=blk, b=b, col=col):
                    o2n = smallp.tile([P, A], BF16, tag="o2n", name="o2n")
                    nc.scalar.activation(o2n[:, :], po[:, :], COPY, scale=rc)
                    res_t = smallp.tile([P, A], F32, tag="res_t", name="res_t")
                    nc.sync.dma_start(
                        out=res_t[:, :],
                        in_=res[blk, b, qt * P:(qt + 1) * P, :])
                    osum = smallp.tile([P, A], F32, tag="osum", name="osum")
                    nc.vector.tensor_add(osum[:, :], o1n[:, qt, :], o2n[:, :])
                    out_t = smallp.tile([P, A], F32, tag="out_t", name="out_t")
                    nc.vector.tensor_add(out_t[:, :], osum[:, :], res_t[:, :])
                    nc.sync.dma_start(
                        out=out[b, qt * P:(qt + 1) * P, col * A:(col + 1) * A],
                        in_=out_t[:, :])
                    if blk == 1:
                        av_bf = smallp.tile([P, A], BF16, tag="av_bf", name="av_bf")
                        nc.vector.tensor_copy(av_bf[:, :], out_t[:, :])
                        nc.scalar.dma_start_transpose(
                            out=avT[b][:, :, qt * P:(qt + 1) * P],
                            in_=av_bf[:, :])

                attention(q2T, k1T, v1, writer1)
                attention(q1T, k2T, v2, writer2)

        wk = load_w(18)
        wq = load_w(19)
        wv = load_w(20)
        for b in range(BLOC):
            xT = load_mt("txt", b, 1)
            kTc = proj_T(wk, xT, "k1T")
            qTc = proj_T(wq, avT[b], "q2T")
            vc = proj_N(wv, xT, "v1")

            def writer_c(qt, po, rc, b=b):
                out_t = smallp.tile([P, A], F32, tag="out_t", name="out_tc")
                nc.scalar.activation(out_t[:, :], po[:, :], COPY, scale=rc)
                nc.sync.dma_start(
                    out=out[b, qt * P:(qt + 1) * P, 3 * A:4 * A],
                    in_=out_t[:, :])

            attention(qTc, kTc, vc, writer_c)


_nc_cache = None
last_results = None


def _get_nc():
    global _nc_cache
    if _nc_cache is None:
        _nc_cache = _build()
    return _nc_cache


def kernel(**inputs):
    global last_results
    txt = np.asarray(inputs["txt"], dtype=np.float32)
    au = np.asarray(inputs["au"], dtype=np.float32)
    vi = np.asarray(inputs["vi"], dtype=np.float32)

    nat = {"txt": txt, "au": au, "vi": vi}
    mt = {n: np.ascontiguousarray(v.transpose(0, 2, 1)).astype(ml_dtypes.bfloat16)
          for n, v in nat.items()}
    wt_all = np.ascontiguousarray(
        np.stack([np.asarray(inputs[n], dtype=np.float32).T for n in W_NAMES])
    ).astype(ml_dtypes.bfloat16)
    res_all = np.stack([txt + au, vi + au, txt + vi])

    in_maps = []
    for c in range(NCORES):
        sl = slice(c * BLOC, (c + 1) * BLOC)
        in_maps.append({
            "mt_txt": mt["txt"][sl],
            "mt_au": mt["au"][sl],
            "mt_vi": mt["vi"][sl],
            "res": np.ascontiguousarray(res_all[:, sl]),
            "wt": wt_all,
        })

    nc = _get_nc()
    last_results = run_bass_kernel_spmd(nc, in_maps, core_ids=list(range(NCORES)))
    core_out = np.concatenate(
        [np.asarray(last_results.results[c]["out"]) for c in range(NCORES)], axis=0)
    return np.concatenate([txt, au, vi, core_out], axis=-1).astype(np.float32)



# revision 5
# speedup vs baseline: 1.2262x; 1.2262x over previous
"""v2: E-fused attention (scores = (m2@E)@m1^T, E = Wq^T@Wk host-precomputed),
transposed-scores layout so exp directly yields probsT for PV (no prob DMA
transposes), softmax sums via interleaved N=1 matmuls against a ones vector.
Data-parallel over batch: 2 batches per core on 8 cores."""

import numpy as np
import ml_dtypes

from concourse import bacc, bass, tile, mybir
from concourse.bass_utils import run_bass_kernel_spmd

B, L, D = 16, 1024, 512
A = D
NCORES = 8
BLOC = B // NCORES
P = 128
DC = D // P          # 4 contraction chunks of 128
LT = L // P          # 8 row blocks
KC = L // P          # 8 key blocks
SCALE = float(1.0 / np.sqrt(np.float32(D)))

F32 = mybir.dt.float32
BF16 = mybir.dt.bfloat16
EXP = mybir.ActivationFunctionType.Exp
COPY = mybir.ActivationFunctionType.Copy


def _build():
    nc = bacc.Bacc("TRN2", target_bir_lowering=False, debug=False,
                   num_devices=NCORES)

    mt_txt = nc.dram_tensor("mt_txt", (BLOC, D, L), BF16, kind="ExternalInput").ap()
    mt_au = nc.dram_tensor("mt_au", (BLOC, D, L), BF16, kind="ExternalInput").ap()
    mt_vi = nc.dram_tensor("mt_vi", (BLOC, D, L), BF16, kind="ExternalInput").ap()
    res = nc.dram_tensor("res", (3, BLOC, L, D), F32, kind="ExternalInput").ap()
    wt = nc.dram_tensor("wt", (14, D, A), BF16, kind="ExternalInput").ap()
    out = nc.dram_tensor("out", (BLOC, L, 4 * A), F32, kind="ExternalOutput").ap()

    with tile.TileContext(nc) as tc:
        _body(nc, tc, mt_txt, mt_au, mt_vi, res, wt, out)

    nc.compile()
    return nc


def _body(nc, tc, mt_txt, mt_au, mt_vi, res, wt, out):
    mt_dram = {"txt": mt_txt, "au": mt_au, "vi": mt_vi}

    with (
        tc.tile_pool(name="persist", bufs=1) as persist,
        tc.tile_pool(name="wpool", bufs=2) as wpool,
        tc.tile_pool(name="work", bufs=2) as work,
        tc.tile_pool(name="small", bufs=3) as smallp,
        tc.tile_pool(name="ps_score", bufs=2, space=bass.MemorySpace.PSUM) as psA,
        tc.tile_pool(name="ps_mm", bufs=3, space=bass.MemorySpace.PSUM) as psB,
        tc.tile_pool(name="ps_sums", bufs=1, space=bass.MemorySpace.PSUM) as psC,
    ):
        # --- persistent tiles -------------------------------------------------
        mtT = {}
        for name in ("txt", "au", "vi"):
            for b in range(BLOC):
                t = persist.tile([P, DC, L], BF16, tag=f"mt_{name}{b}",
                                 name=f"mt_{name}{b}")
                nc.sync.dma_start(
                    out=t[:, :, :],
                    in_=mt_dram[name][b].rearrange("(dc p) l -> p dc l", p=P))
                mtT[(name, b)] = t
        avT = [persist.tile([P, DC, L], BF16, tag=f"avT{b}", name=f"avT{b}")
               for b in range(BLOC)]
        ones = persist.tile([P, 1], BF16, tag="ones", name="ones")
        nc.vector.memset(ones[:, :], 1.0)

        def load_w(j, tag):
            t = wpool.tile([P, DC, A], BF16, tag=tag, name=f"w{j}")
            nc.sync.dma_start(out=t[:, :, :],
                              in_=wt[j].rearrange("(dc p) a -> p dc a", p=P))
            return t

        # --- per-unit phases --------------------------------------------------
        def proj_T(E, mqT):
            """tT[d1-part, q] = (mq @ E)^T ; E tile [d2-part, dc, d1]."""
            tT = work.tile([P, DC, L], BF16, tag="tT", name="tT")
            for db in range(DC):
                for qh in range(2):
                    ps = psB.tile([P, 512], F32, tag="mm", name="ps_pt")
                    for dc in range(DC):
                        nc.tensor.matmul(
                            ps[:, :],
                            E[:, dc, db * P:(db + 1) * P],
                            mqT[:, dc, qh * 512:(qh + 1) * 512],
                            start=(dc == 0), stop=(dc == DC - 1))
                    nc.vector.tensor_copy(tT[:, db, qh * 512:(qh + 1) * 512],
                                          ps[:, :])
            return tT

        def proj_N(WvT, mkvT):
            """v[k-part, a] = mkv @ Wv^T ; WvT tile [d-part, dc, a]."""
            v = work.tile([P, KC, A], BF16, tag="v", name="v")
            for lt in range(LT):
                ps = psB.tile([P, A], F32, tag="mm", name="ps_pn")
                for dc in range(DC):
                    nc.tensor.matmul(ps[:, :],
                                     mkvT[:, dc, lt * P:(lt + 1) * P],
                                     WvT[:, dc, :],
                                     start=(dc == 0), stop=(dc == DC - 1))
                nc.vector.tensor_copy(v[:, lt, :], ps[:, :])
            return v

        def scores_exp(mkvT, tT):
            """probsT[k-part, kc, q] = exp(SCALE * (tT^T contracted with mkvT))."""
            probsT = work.tile([P, KC, L], BF16, tag="probsT", name="probsT")
            for kt in range(KC):
                ps = psA.tile([P, L], F32, tag="score", name="ps_sc")
                for qh in range(2):
                    for dc in range(DC):
                        nc.tensor.matmul(
                            ps[:, qh * 512:(qh + 1) * 512],
                            mkvT[:, dc, kt * P:(kt + 1) * P],
                            tT[:, dc, qh * 512:(qh + 1) * 512],
                            start=(dc == 0), stop=(dc == DC - 1))
                nc.scalar.activation(probsT[:, kt, :], ps[:, :], EXP, scale=SCALE)
            return probsT

        def pv(probsT, v, writer):
            """po[q-part, a] += probs @ v ; sums[q-part] via ones rhs."""
            sums = psC.tile([P, LT], F32, tag="sums", name="sums")
            for qt in range(LT):
                po = psB.tile([P, A], F32, tag="mm", name="ps_pv")
                for kc in range(KC):
                    w = probsT[:, kc, qt * P:(qt + 1) * P]
                    nc.tensor.matmul(po[:, :], w, v[:, kc, :],
                                     start=(kc == 0), stop=(kc == KC - 1))
                    nc.tensor.matmul(sums[:, qt:qt + 1], w, ones[:, :],
                                     start=(kc == 0), stop=(kc == KC - 1))
                recip = smallp.tile([P, 1], F32, tag="recip", name="recip")
                nc.vector.reciprocal(recip[:, :], sums[:, qt:qt + 1])
                writer(qt, po, recip)

        # --- writers ----------------------------------------------------------
        def make_writer1(o1n):
            def writer1(qt, po, rc):
                nc.scalar.activation(o1n[:, qt, :], po[:, :], COPY, scale=rc[:, :])
            return writer1

        def make_writer2(o1n, blk, b, col):
            def writer2(qt, po, rc):
                o2n = smallp.tile([P, A], BF16, tag="o2n", name="o2n")
                nc.scalar.activation(o2n[:, :], po[:, :], COPY, scale=rc[:, :])
                res_t = smallp.tile([P, A], F32, tag="res_t", name="res_t")
                nc.sync.dma_start(out=res_t[:, :],
                                  in_=res[blk, b, qt * P:(qt + 1) * P, :])
                osum = smallp.tile([P, A], F32, tag="osum", name="osum")
                nc.vector.tensor_add(osum[:, :], o2n[:, :], o1n[:, qt, :])
                out_t = smallp.tile([P, A], F32, tag="out_t", name="out_t")
                nc.vector.tensor_add(out_t[:, :], osum[:, :], res_t[:, :])
                nc.sync.dma_start(
                    out=out[b, qt * P:(qt + 1) * P, col * A:(col + 1) * A],
                    in_=out_t[:, :])
                if blk == 1:
                    av_bf = smallp.tile([P, A], BF16, tag="av_bf", name="av_bf")
                    nc.vector.tensor_copy(av_bf[:, :], out_t[:, :])
                    nc.sync.dma_start_transpose(
                        out=avT[b][:, :, qt * P:(qt + 1) * P],
                        in_=av_bf[:, :])
            return writer2

        def make_writer_c(b):
            def writer_c(qt, po, rc):
                out_t = smallp.tile([P, A], F32, tag="out_t", name="out_tc")
                nc.scalar.activation(out_t[:, :], po[:, :], COPY, scale=rc[:, :])
                nc.sync.dma_start(
                    out=out[b, qt * P:(qt + 1) * P, 3 * A:4 * A],
                    in_=out_t[:, :])
            return writer_c

        # --- unit schedule ----------------------------------------------------
        # blocks: (idx, kv/m1, q/m2, out col)
        blocks = [(0, "txt", "au", 0), (1, "vi", "au", 2), (2, "txt", "vi", 1)]

        units = []  # (mqT, mkvT, E_getter, Wv_getter, writer_factory)
        wtiles = {}

        def use_w(j, tag):
            if j not in wtiles:
                wtiles[j] = load_w(j, tag)
            return wtiles[j]

        # prefetch block 0 weights
        for j, tag in ((0, "E1"), (1, "E2"), (2, "Wv1"), (3, "Wv2")):
            use_w(j, tag)

        for blk, n1, n2, col in blocks:
            j0 = blk * 4
            for b in range(BLOC):
                m1T = mtT[(n1, b)]
                m2T = mtT[(n2, b)]
                o1n = work.tile([P, LT, A], BF16, tag="o1n", name="o1n")
                units.append(dict(
                    mqT=m2T, mkvT=m1T, wE=(j0 + 0, "E1"), wV=(j0 + 2, "Wv1"),
                    writer=make_writer1(o1n), prefetch=None))
                units.append(dict(
                    mqT=m1T, mkvT=m2T, wE=(j0 + 1, "E2"), wV=(j0 + 3, "Wv2"),
                    writer=make_writer2(o1n, blk, b, col), prefetch=None))
            # prefetch next block's weights once this block's first unit is queued
            if blk < 2:
                nj = (blk + 1) * 4
                units[-4]["prefetch"] = [(nj + 0, "E1"), (nj + 1, "E2"),
                                         (nj + 2, "Wv1"), (nj + 3, "Wv2")]
            else:
                units[-4]["prefetch"] = [(12, "E1"), (13, "Wv1")]
        for b in range(BLOC):
            units.append(dict(
                mqT=avT[b], mkvT=mtT[("txt", b)], wE=(12, "E1"), wV=(13, "Wv1"),
                writer=make_writer_c(b), prefetch=None))

        # --- software pipeline: PV of unit i-1 between scores and PV of i ----
        pending = None  # (probsT, v, writer)
        for u in units:
            if u["prefetch"]:
                for j, tag in u["prefetch"]:
                    use_w(j, tag)
            E = use_w(*u["wE"])
            Wv = use_w(*u["wV"])
            tT = proj_T(E, u["mqT"])
            v = proj_N(Wv, u["mkvT"])
            probsT = scores_exp(u["mkvT"], tT)
            if pending is not None:
                pv(*pending)
            pending = (probsT, v, u["writer"])
        pv(*pending)


_nc_cache = None
last_results = None


def _get_nc():
    global _nc_cache
    if _nc_cache is None:
        _nc_cache = _build()
    return _nc_cache


def kernel(**inputs):
    global last_results
    txt = np.asarray(inputs["txt"], dtype=np.float32)
    au = np.asarray(inputs["au"], dtype=np.float32)
    vi = np.asarray(inputs["vi"], dtype=np.float32)

    nat = {"txt": txt, "au": au, "vi": vi}
    mt = {n: np.ascontiguousarray(v.transpose(0, 2, 1)).astype(ml_dtypes.bfloat16)
          for n, v in nat.items()}

    g = {n: np.asarray(inputs[n], dtype=np.float32) for n in inputs}
    # E = Wq^T @ Wk per attention; WvT = Wv^T  (all f32 on host, cast bf16)
    wlist = []
    for blk in ("ta", "va", "tv"):
        wlist += [
            g[f"{blk}_qy"].T @ g[f"{blk}_kx"],   # E1 (q-side = m2)
            g[f"{blk}_qx"].T @ g[f"{blk}_ky"],   # E2 (q-side = m1)
            g[f"{blk}_vx"].T,                    # Wv1T (kv = m1)
            g[f"{blk}_vy"].T,                    # Wv2T (kv = m2)
        ]
    wlist += [g["tav_q"].T @ g["tav_k"], g["tav_v"].T]
    wt_all = np.ascontiguousarray(np.stack(wlist)).astype(ml_dtypes.bfloat16)

    res_all = np.stack([txt + au, vi + au, txt + vi])

    in_maps = []
    for c in range(NCORES):
        sl = slice(c * BLOC, (c + 1) * BLOC)
        in_maps.append({
            "mt_txt": mt["txt"][sl],
            "mt_au": mt["au"][sl],
            "mt_vi": mt["vi"][sl],
            "res": np.ascontiguousarray(res_all[:, sl]),
            "wt": wt_all,
        })

    nc = _get_nc()
    last_results = run_bass_kernel_spmd(nc, in_maps, core_ids=list(range(NCORES)))
    core_out = np.concatenate(
        [np.asarray(last_results.results[c]["out"]) for c in range(NCORES)], axis=0)
    return np.concatenate([txt, au, vi, core_out], axis=-1).astype(np.float32)


# revision 7
# speedup vs baseline: 1.2361x; 1.0081x over previous
"""v3: E-fused attention (scores = (m2@E)@m1^T, E = Wq^T@Wk host-precomputed),
transposed-scores layout (exp directly yields probsT for PV; no prob DMA
transposes), softmax sums via interleaved N=1 matmuls against a ones vector,
PSUM released fast via bf16 copies, writers run from SBUF with adds on gpsimd.
Data-parallel over batch: 2 batches per core on 8 cores."""

import numpy as np
import ml_dtypes

from concourse import bacc, bass, tile, mybir
from concourse.bass_utils import run_bass_kernel_spmd

B, L, D = 16, 1024, 512
A = D
NCORES = 8
BLOC = B // NCORES
P = 128
DC = D // P          # 4 contraction chunks of 128
LT = L // P          # 8 row blocks
KC = L // P          # 8 key blocks
SCALE = float(1.0 / np.sqrt(np.float32(D)))

F32 = mybir.dt.float32
BF16 = mybir.dt.bfloat16
EXP = mybir.ActivationFunctionType.Exp
COPY = mybir.ActivationFunctionType.Copy


def _build():
    nc = bacc.Bacc("TRN2", target_bir_lowering=False, debug=False,
                   num_devices=NCORES)

    mt_txt = nc.dram_tensor("mt_txt", (BLOC, D, L), BF16, kind="ExternalInput").ap()
    mt_au = nc.dram_tensor("mt_au", (BLOC, D, L), BF16, kind="ExternalInput").ap()
    mt_vi = nc.dram_tensor("mt_vi", (BLOC, D, L), BF16, kind="ExternalInput").ap()
    res = nc.dram_tensor("res", (3, BLOC, L, D), F32, kind="ExternalInput").ap()
    wt = nc.dram_tensor("wt", (14, D, A), BF16, kind="ExternalInput").ap()
    out = nc.dram_tensor("out", (BLOC, L, 4 * A), F32, kind="ExternalOutput").ap()

    with tile.TileContext(nc) as tc:
        _body(nc, tc, mt_txt, mt_au, mt_vi, res, wt, out)

    nc.compile()
    return nc


def _body(nc, tc, mt_txt, mt_au, mt_vi, res, wt, out):
    mt_dram = {"txt": mt_txt, "au": mt_au, "vi": mt_vi}

    with (
        tc.tile_pool(name="persist", bufs=1) as persist,
        tc.tile_pool(name="wpool", bufs=2) as wpool,
        tc.tile_pool(name="work", bufs=2) as work,
        tc.tile_pool(name="small", bufs=3) as smallp,
        tc.tile_pool(name="ps_score", bufs=2, space=bass.MemorySpace.PSUM) as psA,
        tc.tile_pool(name="ps_mm", bufs=3, space=bass.MemorySpace.PSUM) as psB,
        tc.tile_pool(name="ps_sums", bufs=1, space=bass.MemorySpace.PSUM) as psC,
    ):
        # --- persistent tiles; load first-needed inputs first ----------------
        mtT = {}

        def load_mt(name, b):
            t = persist.tile([P, DC, L], BF16, tag=f"mt_{name}{b}",
                             name=f"mt_{name}{b}")
            nc.sync.dma_start(
                out=t[:, :, :],
                in_=mt_dram[name][b].rearrange("(dc p) l -> p dc l", p=P))
            mtT[(name, b)] = t

        wtiles = {}

        def use_w(j, tag):
            if j not in wtiles:
                t = wpool.tile([P, DC, A], BF16, tag=tag, name=f"w{j}")
                nc.sync.dma_start(out=t[:, :, :],
                                  in_=wt[j].rearrange("(dc p) a -> p dc a", p=P))
                wtiles[j] = t
            return wtiles[j]

        load_mt("txt", 0)
        load_mt("au", 0)
        for j, tag in ((0, "E1"), (2, "Wv1"), (1, "E2"), (3, "Wv2")):
            use_w(j, tag)
        load_mt("vi", 0)
        load_mt("txt", 1)
        load_mt("au", 1)
        load_mt("vi", 1)

        avT = [persist.tile([P, DC, L], BF16, tag=f"avT{b}", name=f"avT{b}")
               for b in range(BLOC)]
        ones = persist.tile([P, 1], BF16, tag="ones", name="ones")
        nc.vector.memset(ones[:, :], 1.0)

        # --- per-unit phases --------------------------------------------------
        def proj_T(E, mqT):
            """tT[d1-part, q] = (mq @ E)^T ; E tile [d2-part, dc, d1]."""
            tT = work.tile([P, DC, L], BF16, tag="tT", name="tT")
            for db in range(DC):
                for qh in range(2):
                    ps = psB.tile([P, 512], F32, tag="mm", name="ps_pt")
                    for dc in range(DC):
                        nc.tensor.matmul(
                            ps[:, :],
                            E[:, dc, db * P:(db + 1) * P],
                            mqT[:, dc, qh * 512:(qh + 1) * 512],
                            start=(dc == 0), stop=(dc == DC - 1))
                    nc.vector.tensor_copy(tT[:, db, qh * 512:(qh + 1) * 512],
                                          ps[:, :])
            return tT

        def proj_N(WvT, mkvT):
            """v[k-part, a] = mkv @ Wv^T ; WvT tile [d-part, dc, a]."""
            v = work.tile([P, KC, A], BF16, tag="v", name="v")
            for lt in range(LT):
                ps = psB.tile([P, A], F32, tag="mm", name="ps_pn")
                for dc in range(DC):
                    nc.tensor.matmul(ps[:, :],
                                     mkvT[:, dc, lt * P:(lt + 1) * P],
                                     WvT[:, dc, :],
                                     start=(dc == 0), stop=(dc == DC - 1))
                nc.vector.tensor_copy(v[:, lt, :], ps[:, :])
            return v

        def scores_exp(mkvT, tT):
            """probsT[k-part, kc, q] = exp(SCALE * scoresT)."""
            probsT = work.tile([P, KC, L], BF16, tag="probsT", name="probsT")
            for kt in range(KC):
                ps = psA.tile([P, L], F32, tag="score", name="ps_sc")
                for qh in range(2):
                    for dc in range(DC):
                        nc.tensor.matmul(
                            ps[:, qh * 512:(qh + 1) * 512],
                            mkvT[:, dc, kt * P:(kt + 1) * P],
                            tT[:, dc, qh * 512:(qh + 1) * 512],
                            start=(dc == 0), stop=(dc == DC - 1))
                nc.scalar.activation(probsT[:, kt, :], ps[:, :], EXP, scale=SCALE)
            return probsT

        def pv(probsT, v, pou, recip):
            """pou[q-part, qt, a] (bf16, unnormalized), recip[q-part, qt]."""
            sums = psC.tile([P, LT], F32, tag="sums", name="sums")
            for qt in range(LT):
                po = psB.tile([P, A], F32, tag="mm", name="ps_pv")
                for kc in range(KC):
                    w = probsT[:, kc, qt * P:(qt + 1) * P]
                    nc.tensor.matmul(po[:, :], w, v[:, kc, :],
                                     start=(kc == 0), stop=(kc == KC - 1))
                    nc.tensor.matmul(sums[:, qt:qt + 1], w, ones[:, :],
                                     start=(kc == 0), stop=(kc == KC - 1))
                nc.vector.reciprocal(recip[:, qt:qt + 1], sums[:, qt:qt + 1])
                nc.vector.tensor_copy(pou[:, qt, :], po[:, :])

        # --- combine writers (SBUF only; adds on gpsimd) ---------------------
        def combine_pair(pou1, rc1, pou2, rc2, blk, b, col):
            for qt in range(LT):
                t1 = smallp.tile([P, A], F32, tag="t1", name="t1", bufs=2)
                nc.scalar.activation(t1[:, :], pou1[:, qt, :], COPY,
                                     scale=rc1[:, qt:qt + 1])
                o2 = smallp.tile([P, A], F32, tag="o2", name="o2", bufs=2)
                nc.vector.tensor_scalar_mul(o2[:, :], pou2[:, qt, :],
                                            rc2[:, qt:qt + 1])
                res_t = smallp.tile([P, A], F32, tag="res_t", name="res_t")
                nc.sync.dma_start(out=res_t[:, :],
                                  in_=res[blk, b, qt * P:(qt + 1) * P, :])
                osum = smallp.tile([P, A], F32, tag="osum", name="osum")
                nc.gpsimd.tensor_add(osum[:, :], t1[:, :], o2[:, :])
                nc.gpsimd.tensor_add(osum[:, :], osum[:, :], res_t[:, :])
                nc.sync.dma_start(
                    out=out[b, qt * P:(qt + 1) * P, col * A:(col + 1) * A],
                    in_=osum[:, :])
                if blk == 1:
                    av_bf = smallp.tile([P, A], BF16, tag="av_bf", name="av_bf",
                                        bufs=2)
                    nc.vector.tensor_copy(av_bf[:, :], osum[:, :])
                    nc.sync.dma_start_transpose(
                        out=avT[b][:, :, qt * P:(qt + 1) * P],
                        in_=av_bf[:, :])

        def combine_cross(pou, rc, b):
            for qt in range(LT):
                out_t = smallp.tile([P, A], F32, tag="osum", name="out_tc")
                nc.scalar.activation(out_t[:, :], pou[:, qt, :], COPY,
                                     scale=rc[:, qt:qt + 1])
                nc.sync.dma_start(
                    out=out[b, qt * P:(qt + 1) * P, 3 * A:4 * A],
                    in_=out_t[:, :])

        # --- unit schedule ----------------------------------------------------
        # blocks: (idx, kv/m1, q/m2, out col)
        blocks = [(0, "txt", "au", 0), (1, "vi", "au", 2), (2, "txt", "vi", 1)]

        units = []
        for blk, n1, n2, col in blocks:
            j0 = blk * 4
            for b in range(BLOC):
                units.append(dict(
                    mq=(n2, b), mkv=(n1, b), wE=(j0 + 0, "E1"),
                    wV=(j0 + 2, "Wv1"), kind="attn1", prefetch=None))
                units.append(dict(
                    mq=(n1, b), mkv=(n2, b), wE=(j0 + 1, "E2"),
                    wV=(j0 + 3, "Wv2"), kind=("attn2", blk, b, col),
                    prefetch=None))
            if blk < 2:
                nj = (blk + 1) * 4
                units[-4]["prefetch"] = [(nj + 0, "E1"), (nj + 1, "E2"),
                                         (nj + 2, "Wv1"), (nj + 3, "Wv2")]
            else:
                units[-4]["prefetch"] = [(12, "E1"), (13, "Wv1")]
        for b in range(BLOC):
            units.append(dict(
                mq=("avT", b), mkv=("txt", b), wE=(12, "E1"), wV=(13, "Wv1"),
                kind=("cross", b), prefetch=None))

        # --- software pipeline: PV+combine of unit i-1 inside unit i ---------
        pending = None          # (probsT, v, pou, recip, kind)
        prev_attn1 = None       # (pou, recip) of the attn1 of current pair

        def flush(p):
            nonlocal prev_attn1
            probsT, v, pou, recip, kind = p
            pv(probsT, v, pou, recip)
            if kind == "attn1":
                prev_attn1 = (pou, recip)
            elif kind[0] == "attn2":
                _, blk, b, col = kind
                pou1, rc1 = prev_attn1
                combine_pair(pou1, rc1, pou, recip, blk, b, col)
            else:
                combine_cross(pou, recip, kind[1])

        for u in units:
            if u["prefetch"]:
                for j, tag in u["prefetch"]:
                    use_w(j, tag)
            E = use_w(*u["wE"])
            Wv = use_w(*u["wV"])
            mq = avT[u["mq"][1]] if u["mq"][0] == "avT" else mtT[u["mq"]]
            mkv = mtT[u["mkv"]]
            tT = proj_T(E, mq)
            v = proj_N(Wv, mkv)
            probsT = scores_exp(mkv, tT)
            pou = work.tile([P, LT, A], BF16, tag="pou", name="pou", bufs=3)
            recip = work.tile([P, LT], F32, tag="recip", name="recip", bufs=3)
            if pending is not None:
                flush(pending)
            pending = (probsT, v, pou, recip, u["kind"])
        flush(pending)


_nc_cache = None
last_results = None


def _get_nc():
    global _nc_cache
    if _nc_cache is None:
        _nc_cache = _build()
    return _nc_cache


def kernel(**inputs):
    global last_results
    txt = np.asarray(inputs["txt"], dtype=np.float32)
    au = np.asarray(inputs["au"], dtype=np.float32)
    vi = np.asarray(inputs["vi"], dtype=np.float32)

    nat = {"txt": txt, "au": au, "vi": vi}
    mt = {n: np.ascontiguousarray(v.transpose(0, 2, 1)).astype(ml_dtypes.bfloat16)
          for n, v in nat.items()}

    g = {n: np.asarray(inputs[n], dtype=np.float32) for n in inputs}
    # E = Wq^T @ Wk per attention; WvT = Wv^T  (all f32 on host, cast bf16)
    wlist = []
    for blk in ("ta", "va", "tv"):
        wlist += [
            g[f"{blk}_qy"].T @ g[f"{blk}_kx"],   # E1 (q-side = m2)
            g[f"{blk}_qx"].T @ g[f"{blk}_ky"],   # E2 (q-side = m1)
            g[f"{blk}_vx"].T,                    # Wv1T (kv = m1)
            g[f"{blk}_vy"].T,                    # Wv2T (kv = m2)
        ]
    wlist += [g["tav_q"].T @ g["tav_k"], g["tav_v"].T]
    wt_all = np.ascontiguousarray(np.stack(wlist)).astype(ml_dtypes.bfloat16)

    res_all = np.stack([txt + au, vi + au, txt + vi])

    in_maps = []
    for c in range(NCORES):
        sl = slice(c * BLOC, (c + 1) * BLOC)
        in_maps.append({
            "mt_txt": mt["txt"][sl],
            "mt_au": mt["au"][sl],
            "mt_vi": mt["vi"][sl],
            "res": np.ascontiguousarray(res_all[:, sl]),
            "wt": wt_all,
        })

    nc = _get_nc()
    last_results = run_bass_kernel_spmd(nc, in_maps, core_ids=list(range(NCORES)))
    core_out = np.concatenate(
        [np.asarray(last_results.results[c]["out"]) for c in range(NCORES)], axis=0)
    return np.concatenate([txt, au, vi, core_out], axis=-1).astype(np.float32)


# revision 12
# speedup vs baseline: 1.3686x; 1.1072x over previous
"""v4: E-fused attention (scores = (m2@E)@m1^T, E = Wq^T@Wk host-precomputed),
transposed-scores layout (exp directly yields probsT for PV; no prob DMA
transposes), softmax sums via interleaved N=1 matmuls vs a ones vector,
PV outputs normalized during the PSUM->SBUF copy (tensor_scalar_mul by the
softmax reciprocal), residual combines deferred two pipeline stages so they
never gate the next unit's scores. Data-parallel: 2 batches/core, 8 cores."""

import numpy as np
import ml_dtypes

from concourse import bacc, bass, tile, mybir
from concourse.bass_utils import run_bass_kernel_spmd

B, L, D = 16, 1024, 512
A = D
NCORES = 8
BLOC = B // NCORES
P = 128
DC = D // P          # 4 contraction chunks of 128
LT = L // P          # 8 row blocks
KC = L // P          # 8 key blocks
SCALE = float(1.0 / np.sqrt(np.float32(D)))

F32 = mybir.dt.float32
BF16 = mybir.dt.bfloat16
EXP = mybir.ActivationFunctionType.Exp
COPY = mybir.ActivationFunctionType.Copy


def _build():
    nc = bacc.Bacc("TRN2", target_bir_lowering=False, debug=False,
                   num_devices=NCORES)

    mt_txt = nc.dram_tensor("mt_txt", (BLOC, D, L), BF16, kind="ExternalInput").ap()
    mt_au = nc.dram_tensor("mt_au", (BLOC, D, L), BF16, kind="ExternalInput").ap()
    mt_vi = nc.dram_tensor("mt_vi", (BLOC, D, L), BF16, kind="ExternalInput").ap()
    res = nc.dram_tensor("res", (3, BLOC, L, D), F32, kind="ExternalInput").ap()
    wt = nc.dram_tensor("wt", (14, D, A), BF16, kind="ExternalInput").ap()
    out = nc.dram_tensor("out", (BLOC, L, 4 * A), F32, kind="ExternalOutput").ap()

    with tile.TileContext(nc) as tc:
        _body(nc, tc, mt_txt, mt_au, mt_vi, res, wt, out)

    nc.compile()
    return nc


def _body(nc, tc, mt_txt, mt_au, mt_vi, res, wt, out):
    mt_dram = {"txt": mt_txt, "au": mt_au, "vi": mt_vi}

    with (
        tc.tile_pool(name="persist", bufs=1) as persist,
        tc.tile_pool(name="wpool", bufs=2) as wpool,
        tc.tile_pool(name="work", bufs=2) as work,
        tc.tile_pool(name="small", bufs=3) as smallp,
        tc.tile_pool(name="ps_score", bufs=2, space=bass.MemorySpace.PSUM) as psA,
        tc.tile_pool(name="ps_mm", bufs=3, space=bass.MemorySpace.PSUM) as psB,
        tc.tile_pool(name="ps_sums", bufs=1, space=bass.MemorySpace.PSUM) as psC,
    ):
        # --- persistent tiles; load first-needed inputs first ----------------
        mtT = {}

        def load_mt(name, b):
            t = persist.tile([P, DC, L], BF16, tag=f"mt_{name}{b}",
                             name=f"mt_{name}{b}")
            nc.sync.dma_start(
                out=t[:, :, :],
                in_=mt_dram[name][b].rearrange("(dc p) l -> p dc l", p=P))
            mtT[(name, b)] = t

        wtiles = {}

        def use_w(j, tag):
            if j not in wtiles:
                t = wpool.tile([P, DC, A], BF16, tag=tag, name=f"w{j}")
                nc.sync.dma_start(out=t[:, :, :],
                                  in_=wt[j].rearrange("(dc p) a -> p dc a", p=P))
                wtiles[j] = t
            return wtiles[j]

        load_mt("txt", 0)
        load_mt("au", 0)
        for j, tag in ((0, "E1"), (2, "Wv1"), (1, "E2"), (3, "Wv2")):
            use_w(j, tag)
        load_mt("vi", 0)
        load_mt("txt", 1)
        load_mt("au", 1)
        load_mt("vi", 1)

        avT = [persist.tile([P, DC, L], BF16, tag=f"avT{b}", name=f"avT{b}")
               for b in range(BLOC)]
        ones = persist.tile([P, 1], BF16, tag="ones", name="ones")
        nc.vector.memset(ones[:, :], 1.0)

        # --- per-unit phases --------------------------------------------------
        def proj_T(E, mqT):
            """tT[d1-part, q] = (mq @ E)^T ; E tile [d2-part, dc, d1]."""
            tT = work.tile([P, DC, L], BF16, tag="tT", name="tT")
            for db in range(DC):
                for qh in range(2):
                    ps = psB.tile([P, 512], F32, tag="mm", name="ps_pt")
                    for dc in range(DC):
                        nc.tensor.matmul(
                            ps[:, :],
                            E[:, dc, db * P:(db + 1) * P],
                            mqT[:, dc, qh * 512:(qh + 1) * 512],
                            start=(dc == 0), stop=(dc == DC - 1))
                    nc.vector.tensor_copy(tT[:, db, qh * 512:(qh + 1) * 512],
                                          ps[:, :])
            return tT

        def proj_N(WvT, mkvT):
            """v[k-part, a] = mkv @ Wv^T ; WvT tile [d-part, dc, a]."""
            v = work.tile([P, KC, A], BF16, tag="v", name="v")
            for lt in range(LT):
                ps = psB.tile([P, A], F32, tag="mm", name="ps_pn")
                for dc in range(DC):
                    nc.tensor.matmul(ps[:, :],
                                     mkvT[:, dc, lt * P:(lt + 1) * P],
                                     WvT[:, dc, :],
                                     start=(dc == 0), stop=(dc == DC - 1))
                nc.scalar.activation(v[:, lt, :], ps[:, :], COPY)
            return v

        def scores_exp(mkvT, tT):
            """probsT[k-part, kc, q] = exp(SCALE * scoresT)."""
            probsT = work.tile([P, KC, L], BF16, tag="probsT", name="probsT")
            for kt in range(KC):
                ps = psA.tile([P, L], F32, tag="score", name="ps_sc")
                for qh in range(2):
                    for dc in range(DC):
                        nc.tensor.matmul(
                            ps[:, qh * 512:(qh + 1) * 512],
                            mkvT[:, dc, kt * P:(kt + 1) * P],
                            tT[:, dc, qh * 512:(qh + 1) * 512],
                            start=(dc == 0), stop=(dc == DC - 1))
                nc.scalar.activation(probsT[:, kt, :], ps[:, :], EXP, scale=SCALE)
            return probsT

        def pv_range(probsT, v, pou, recip, sums, qts, cross_b=None):
            """pou[q-part, qt, a] = (probs @ v) * recip  (normalized, bf16).
            For cross units, writes the output column directly instead."""
            for qt in qts:
                po = psB.tile([P, A], F32, tag="mm", name="ps_pv")
                for kc in range(KC):
                    w = probsT[:, kc, qt * P:(qt + 1) * P]
                    nc.tensor.matmul(po[:, :], w, v[:, kc, :],
                                     start=(kc == 0), stop=(kc == KC - 1))
                    nc.tensor.matmul(sums[:, qt:qt + 1], w, ones[:, :],
                                     start=(kc == 0), stop=(kc == KC - 1))
                nc.vector.reciprocal(recip[:, qt:qt + 1], sums[:, qt:qt + 1])
                if cross_b is None:
                    nc.vector.tensor_scalar_mul(pou[:, qt, :], po[:, :],
                                                recip[:, qt:qt + 1])
                else:
                    out_c = smallp.tile([P, A], F32, tag="out_c", name="out_c")
                    nc.vector.tensor_scalar_mul(out_c[:, :], po[:, :],
                                                recip[:, qt:qt + 1])
                    nc.sync.dma_start(
                        out=out[cross_b, qt * P:(qt + 1) * P, 3 * A:4 * A],
                        in_=out_c[:, :])

        # --- deferred combine: out = pou1 + pou2 + res -----------------------
        def combine_pair(pou1, pou2, blk, b, col):
            for qt in range(LT):
                res_t = smallp.tile([P, A], F32, tag="res_t", name="res_t")
                nc.sync.dma_start(out=res_t[:, :],
                                  in_=res[blk, b, qt * P:(qt + 1) * P, :])
                osum = smallp.tile([P, A], F32, tag="osum", name="osum")
                nc.vector.tensor_add(osum[:, :], pou1[:, qt, :], pou2[:, qt, :])
                nc.vector.tensor_add(osum[:, :], osum[:, :], res_t[:, :])
                nc.sync.dma_start(
                    out=out[b, qt * P:(qt + 1) * P, col * A:(col + 1) * A],
                    in_=osum[:, :])
                if blk == 1:
                    av_bf = smallp.tile([P, A], BF16, tag="av_bf", name="av_bf",
                                        bufs=2)
                    nc.vector.tensor_copy(av_bf[:, :], osum[:, :])
                    nc.sync.dma_start_transpose(
                        out=avT[b][:, :, qt * P:(qt + 1) * P],
                        in_=av_bf[:, :])

        # --- unit schedule ----------------------------------------------------
        # blocks: (idx, kv/m1, q/m2, out col)
        blocks = [(0, "txt", "au", 0), (1, "vi", "au", 2), (2, "txt", "vi", 1)]

        units = []
        for blk, n1, n2, col in blocks:
            j0 = blk * 4
            for b in range(BLOC):
                units.append(dict(
                    mq=(n2, b), mkv=(n1, b), wE=(j0 + 0, "E1"),
                    wV=(j0 + 2, "Wv1"), kind="attn1", prefetch=None))
                units.append(dict(
                    mq=(n1, b), mkv=(n2, b), wE=(j0 + 1, "E2"),
                    wV=(j0 + 3, "Wv2"), kind=("attn2", blk, b, col),
                    prefetch=None))
            if blk < 2:
                nj = (blk + 1) * 4
                units[-4]["prefetch"] = [(nj + 0, "E1"), (nj + 1, "E2"),
                                         (nj + 2, "Wv1"), (nj + 3, "Wv2")]
            else:
                units[-4]["prefetch"] = [(12, "E1"), (13, "Wv1")]
        for b in range(BLOC):
            units.append(dict(
                mq=("avT", b), mkv=("txt", b), wE=(12, "E1"), wV=(13, "Wv1"),
                kind=("cross", b), prefetch=None))

        # --- software pipeline -------------------------------------------
        # Unit step i emits: proj(i), pvA(i-1), scores(i), pvB(i-1), then any
        # residual combine created at step <= i-1 (i.e. lagging one extra unit).
        pend_pv = None        # (probsT, v, pou, recip, kind) awaiting PV
        comb_queue = []       # [(pou1, pou2, kind, created_step)]
        prev_attn1 = None     # pou of the pair's first attention

        def emit_pv(p, half):
            probsT, v, pou, recip, sums, kind = p
            cross_b = kind[1] if (kind != "attn1" and kind[0] == "cross") else None
            qts = range(0, 4) if half == 0 else range(4, LT)
            pv_range(probsT, v, pou, recip, sums, qts, cross_b)

        def finish_pv(p, step):
            nonlocal prev_attn1
            _, _, pou_p, _, _, kind_p = p
            if kind_p == "attn1":
                prev_attn1 = pou_p
            elif kind_p[0] == "attn2":
                comb_queue.append((prev_attn1, pou_p, kind_p, step))

        for step, u in enumerate(units):
            if u["prefetch"]:
                for j, tag in u["prefetch"]:
                    use_w(j, tag)
            E = use_w(*u["wE"])
            Wv = use_w(*u["wV"])
            mq = avT[u["mq"][1]] if u["mq"][0] == "avT" else mtT[u["mq"]]
            mkv = mtT[u["mkv"]]
            tT = proj_T(E, mq)
            v = proj_N(Wv, mkv)
            if pend_pv is not None:
                emit_pv(pend_pv, 0)
            probsT = scores_exp(mkv, tT)
            if pend_pv is not None:
                emit_pv(pend_pv, 1)
                finish_pv(pend_pv, step)
            while comb_queue and comb_queue[0][3] < step:
                pou1, pou2, kind, _ = comb_queue.pop(0)
                combine_pair(pou1, pou2, kind[1], kind[2], kind[3])
            pou = work.tile([P, LT, A], BF16, tag="pou", name="pou", bufs=3)
            recip = work.tile([P, LT], F32, tag="recip", name="recip", bufs=3)
            sums = psC.tile([P, LT], F32, tag="sums", name="sums")
            pend_pv = (probsT, v, pou, recip, sums, u["kind"])
        # tail flush
        emit_pv(pend_pv, 0)
        emit_pv(pend_pv, 1)
        finish_pv(pend_pv, len(units))
        for pou1, pou2, kind, _ in comb_queue:
            combine_pair(pou1, pou2, kind[1], kind[2], kind[3])


_nc_cache = None
last_results = None


def _get_nc():
    global _nc_cache
    if _nc_cache is None:
        _nc_cache = _build()
    return _nc_cache


def kernel(**inputs):
    global last_results
    txt = np.asarray(inputs["txt"], dtype=np.float32)
    au = np.asarray(inputs["au"], dtype=np.float32)
    vi = np.asarray(inputs["vi"], dtype=np.float32)

    nat = {"txt": txt, "au": au, "vi": vi}
    mt = {n: np.ascontiguousarray(v.transpose(0, 2, 1)).astype(ml_dtypes.bfloat16)
          for n, v in nat.items()}

    g = {n: np.asarray(inputs[n], dtype=np.float32) for n in inputs}
    # E = Wq^T @ Wk per attention; WvT = Wv^T  (all f32 on host, cast bf16)
    wlist = []
    for blk in ("ta", "va", "tv"):
        wlist += [
            g[f"{blk}_qy"].T @ g[f"{blk}_kx"],   # E1 (q-side = m2)
            g[f"{blk}_qx"].T @ g[f"{blk}_ky"],   # E2 (q-side = m1)
            g[f"{blk}_vx"].T,                    # Wv1T (kv = m1)
            g[f"{blk}_vy"].T,                    # Wv2T (kv = m2)
        ]
    wlist += [g["tav_q"].T @ g["tav_k"], g["tav_v"].T]
    wt_all = np.ascontiguousarray(np.stack(wlist)).astype(ml_dtypes.bfloat16)

    res_all = np.stack([txt + au, vi + au, txt + vi])

    in_maps = []
    for c in range(NCORES):
        sl = slice(c * BLOC, (c + 1) * BLOC)
        in_maps.append({
            "mt_txt": mt["txt"][sl],
            "mt_au": mt["au"][sl],
            "mt_vi": mt["vi"][sl],
            "res": np.ascontiguousarray(res_all[:, sl]),
            "wt": wt_all,
        })

    nc = _get_nc()
    last_results = run_bass_kernel_spmd(nc, in_maps, core_ids=list(range(NCORES)))
    core_out = np.concatenate(
        [np.asarray(last_results.results[c]["out"]) for c in range(NCORES)], axis=0)
    return np.concatenate([txt, au, vi, core_out], axis=-1).astype(np.float32)


# revision 16
# speedup vs baseline: 2.0294x; 1.4828x over previous
"""v5: mixed-precision variant of v4. The six symmetric attentions run in
fp8(e4m3) DoubleRow (2 k-tiles per matmul instruction, 2x PE rate); the final
cross attention (whose softmax is much sharper) stays on the proven bf16 path.
Host pre-scales the fused E=Wq^T@Wk and Wv weights by 16 to dodge fp8
subnormals; the scale is folded back via exp(scale) and the softmax
reciprocal (ones vector = 16). exp() carries bias=-2 to keep probabilities
inside fp8 range (cancels in the softmax ratio).
Data-parallel: 2 batches/core, 8 cores."""

import numpy as np
import ml_dtypes

from concourse import bacc, bass, tile, mybir
from concourse.bass_utils import run_bass_kernel_spmd

B, L, D = 16, 1024, 512
A = D
NCORES = 8
BLOC = B // NCORES
P = 128
DC = D // P          # 4 contraction chunks of 128
LT = L // P          # 8 row blocks
KC = L // P          # 8 key blocks
SCALE = float(1.0 / np.sqrt(np.float32(D)))
WSC = 16.0           # host weight pre-scale for fp8 (subnormal avoidance)
EB = -2.0            # exp bias: probs scaled by e^-2, cancels in softmax

F32 = mybir.dt.float32
BF16 = mybir.dt.bfloat16
FP8 = mybir.dt.float8e4
DR = mybir.MatmulPerfMode.DoubleRow
EXP = mybir.ActivationFunctionType.Exp
COPY = mybir.ActivationFunctionType.Copy


def _build():
    nc = bacc.Bacc("TRN2", target_bir_lowering=False, debug=False,
                   num_devices=NCORES)

    mt_txt = nc.dram_tensor("mt_txt", (BLOC, D, L), FP8, kind="ExternalInput").ap()
    mt_au = nc.dram_tensor("mt_au", (BLOC, D, L), FP8, kind="ExternalInput").ap()
    mt_vi = nc.dram_tensor("mt_vi", (BLOC, D, L), FP8, kind="ExternalInput").ap()
    mt_txtb = nc.dram_tensor("mt_txtb", (BLOC, D, L), BF16,
                             kind="ExternalInput").ap()
    res = nc.dram_tensor("res", (3, BLOC, L, D), F32, kind="ExternalInput").ap()
    wt = nc.dram_tensor("wt", (12, D, A), FP8, kind="ExternalInput").ap()
    wtb = nc.dram_tensor("wtb", (2, D, A), BF16, kind="ExternalInput").ap()
    out = nc.dram_tensor("out", (BLOC, L, 4 * A), F32, kind="ExternalOutput").ap()

    with tile.TileContext(nc) as tc:
        _body(nc, tc, mt_txt, mt_au, mt_vi, mt_txtb, res, wt, wtb, out)

    nc.compile()
    return nc


def _body(nc, tc, mt_txt, mt_au, mt_vi, mt_txtb, res, wt, wtb, out):
    mt_dram = {"txt": mt_txt, "au": mt_au, "vi": mt_vi}

    with (
        tc.tile_pool(name="persist", bufs=1) as persist,
        tc.tile_pool(name="wpool", bufs=2) as wpool,
        tc.tile_pool(name="work", bufs=2) as work,
        tc.tile_pool(name="small", bufs=3) as smallp,
        tc.tile_pool(name="ps_score", bufs=2, space=bass.MemorySpace.PSUM) as psA,
        tc.tile_pool(name="ps_mm", bufs=3, space=bass.MemorySpace.PSUM) as psB,
        tc.tile_pool(name="ps_sums", bufs=1, space=bass.MemorySpace.PSUM) as psC,
    ):
        # --- persistent tiles; load first-needed inputs first ----------------
        mtT = {}

        def load_mt(name, b):
            t = persist.tile([P, DC, L], FP8, tag=f"mt_{name}{b}",
                             name=f"mt_{name}{b}")
            nc.sync.dma_start(
                out=t[:, :, :],
                in_=mt_dram[name][b].rearrange("(dc p) l -> p dc l", p=P))
            mtT[(name, b)] = t

        wtiles = {}

        def use_w(j, tag):
            # j >= 12 -> bf16 cross weights from wtb
            if j not in wtiles:
                if j >= 12:
                    t = wpool.tile([P, DC, A], BF16, tag=tag, name=f"w{j}")
                    nc.sync.dma_start(
                        out=t[:, :, :],
                        in_=wtb[j - 12].rearrange("(dc p) a -> p dc a", p=P))
                else:
                    t = wpool.tile([P, DC, A], FP8, tag=tag, name=f"w{j}")
                    nc.sync.dma_start(
                        out=t[:, :, :],
                        in_=wt[j].rearrange("(dc p) a -> p dc a", p=P))
                wtiles[j] = t
            return wtiles[j]

        load_mt("txt", 0)
        load_mt("au", 0)
        for j, tag in ((0, "E1"), (2, "Wv1"), (1, "E2"), (3, "Wv2")):
            use_w(j, tag)
        load_mt("vi", 0)
        load_mt("txt", 1)
        load_mt("au", 1)
        load_mt("vi", 1)
        txtb = []
        for b in range(BLOC):
            t = persist.tile([P, DC, L], BF16, tag=f"txtb{b}", name=f"txtb{b}")
            nc.sync.dma_start(
                out=t[:, :, :],
                in_=mt_txtb[b].rearrange("(dc p) l -> p dc l", p=P))
            txtb.append(t)

        avT = [persist.tile([P, DC, L], BF16, tag=f"avT{b}", name=f"avT{b}")
               for b in range(BLOC)]
        onesDR = persist.tile([P, 2, 1], FP8, tag="onesDR", name="onesDR")
        nc.vector.memset(onesDR[:, :, :], WSC)
        ones_bf = persist.tile([P, 1], BF16, tag="ones_bf", name="ones_bf")
        nc.vector.memset(ones_bf[:, :], 1.0)
        ebias = persist.tile([P, 1], F32, tag="ebias", name="ebias")
        nc.vector.memset(ebias[:, :], EB)

        # --- per-unit phases --------------------------------------------------
        def proj_T(E, mqT, fp8):
            """tT[d1-part, q] = (mq @ E)^T ; E tile [d2-part, dc, d1]."""
            tT = work.tile([P, DC, L], FP8 if fp8 else BF16, tag="tT", name="tT")
            for db in range(DC):
                for qh in range(2):
                    ps = psB.tile([P, 512], F32, tag="mm", name="ps_pt")
                    if fp8:
                        for dcp in (0, 2):
                            nc.tensor.matmul(
                                ps[:, :],
                                E[:, dcp:dcp + 2, db * P:(db + 1) * P],
                                mqT[:, dcp:dcp + 2, qh * 512:(qh + 1) * 512],
                                start=(dcp == 0), stop=(dcp == 2), perf_mode=DR)
                    else:
                        for dc in range(DC):
                            nc.tensor.matmul(
                                ps[:, :],
                                E[:, dc, db * P:(db + 1) * P],
                                mqT[:, dc, qh * 512:(qh + 1) * 512],
                                start=(dc == 0), stop=(dc == DC - 1))
                    nc.vector.tensor_copy(tT[:, db, qh * 512:(qh + 1) * 512],
                                          ps[:, :])
            return tT

        def proj_N(WvT, mkvT, fp8):
            """v[k-part, a] = mkv @ Wv^T ; WvT tile [d-part, dc, a]."""
            v = work.tile([P, KC, A], FP8 if fp8 else BF16, tag="v", name="v")
            for lt in range(LT):
                ps = psB.tile([P, A], F32, tag="mm", name="ps_pn")
                if fp8:
                    for dcp in (0, 2):
                        nc.tensor.matmul(ps[:, :],
                                         mkvT[:, dcp:dcp + 2, lt * P:(lt + 1) * P],
                                         WvT[:, dcp:dcp + 2, :],
                                         start=(dcp == 0), stop=(dcp == 2),
                                         perf_mode=DR)
                else:
                    for dc in range(DC):
                        nc.tensor.matmul(ps[:, :],
                                         mkvT[:, dc, lt * P:(lt + 1) * P],
                                         WvT[:, dc, :],
                                         start=(dc == 0), stop=(dc == DC - 1))
                nc.scalar.activation(v[:, lt, :], ps[:, :], COPY)
            return v

        def scores_exp(mkvT, tT, fp8):
            """probsT[k-part, kc, q] = exp(scale * scoresT + EB)."""
            probsT = work.tile([P, KC, L], FP8 if fp8 else BF16, tag="probsT",
                               name="probsT")
            sc = SCALE / WSC if fp8 else SCALE
            for kt in range(KC):
                ps = psA.tile([P, L], F32, tag="score", name="ps_sc")
                for qh in range(2):
                    if fp8:
                        for dcp in (0, 2):
                            nc.tensor.matmul(
                                ps[:, qh * 512:(qh + 1) * 512],
                                mkvT[:, dcp:dcp + 2, kt * P:(kt + 1) * P],
                                tT[:, dcp:dcp + 2, qh * 512:(qh + 1) * 512],
                                start=(dcp == 0), stop=(dcp == 2), perf_mode=DR)
                    else:
                        for dc in range(DC):
                            nc.tensor.matmul(
                                ps[:, qh * 512:(qh + 1) * 512],
                                mkvT[:, dc, kt * P:(kt + 1) * P],
                                tT[:, dc, qh * 512:(qh + 1) * 512],
                                start=(dc == 0), stop=(dc == DC - 1))
                nc.scalar.activation(probsT[:, kt, :], ps[:, :], EXP,
                                     scale=sc, bias=ebias[:, :])
            return probsT

        def pv_range(probsT, v, pou, recip, sums, qts, fp8, cross_b=None):
            """pou[q-part, qt, a] = (probs @ v) * recip  (normalized, bf16)."""
            for qt in qts:
                po = psB.tile([P, A], F32, tag="mm", name="ps_pv")
                if fp8:
                    for kcp in (0, 2, 4, 6):
                        w = probsT[:, kcp:kcp + 2, qt * P:(qt + 1) * P]
                        nc.tensor.matmul(po[:, :], w, v[:, kcp:kcp + 2, :],
                                         start=(kcp == 0), stop=(kcp == 6),
                                         perf_mode=DR)
                        nc.tensor.matmul(sums[:, qt:qt + 1], w, onesDR[:, :, :],
                                         start=(kcp == 0), stop=(kcp == 6),
                                         perf_mode=DR)
                else:
                    for kc in range(KC):
                        w = probsT[:, kc, qt * P:(qt + 1) * P]
                        nc.tensor.matmul(po[:, :], w, v[:, kc, :],
                                         start=(kc == 0), stop=(kc == KC - 1))
                        nc.tensor.matmul(sums[:, qt:qt + 1], w, ones_bf[:, :],
                                         start=(kc == 0), stop=(kc == KC - 1))
                nc.vector.reciprocal(recip[:, qt:qt + 1], sums[:, qt:qt + 1])
                if cross_b is None:
                    nc.vector.tensor_scalar_mul(pou[:, qt, :], po[:, :],
                                                recip[:, qt:qt + 1])
                else:
                    out_c = smallp.tile([P, A], F32, tag="out_c", name="out_c")
                    nc.vector.tensor_scalar_mul(out_c[:, :], po[:, :],
                                                recip[:, qt:qt + 1])
                    nc.sync.dma_start(
                        out=out[cross_b, qt * P:(qt + 1) * P, 3 * A:4 * A],
                        in_=out_c[:, :])

        # --- deferred combine: out = pou1 + pou2 + res -----------------------
        def combine_pair(pou1, pou2, blk, b, col):
            for qt in range(LT):
                res_t = smallp.tile([P, A], F32, tag="res_t", name="res_t")
                nc.sync.dma_start(out=res_t[:, :],
                                  in_=res[blk, b, qt * P:(qt + 1) * P, :])
                osum = smallp.tile([P, A], F32, tag="osum", name="osum")
                if blk == 1:
                    nc.vector.tensor_add(osum[:, :], pou1[:, qt, :],
                                         pou2[:, qt, :])
                    nc.vector.tensor_add(osum[:, :], osum[:, :], res_t[:, :])
                else:
                    nc.gpsimd.tensor_add(osum[:, :], pou1[:, qt, :],
                                         pou2[:, qt, :])
                    nc.gpsimd.tensor_add(osum[:, :], osum[:, :], res_t[:, :])
                nc.sync.dma_start(
                    out=out[b, qt * P:(qt + 1) * P, col * A:(col + 1) * A],
                    in_=osum[:, :])
                if blk == 1:
                    av_bf = smallp.tile([P, A], BF16, tag="av_bf", name="av_bf",
                                        bufs=2)
                    nc.vector.tensor_copy(av_bf[:, :], osum[:, :])
                    nc.sync.dma_start_transpose(
                        out=avT[b][:, :, qt * P:(qt + 1) * P],
                        in_=av_bf[:, :])

        # --- unit schedule ----------------------------------------------------
        blocks = [(0, "txt", "au", 0), (1, "vi", "au", 2), (2, "txt", "vi", 1)]

        units = []
        for blk, n1, n2, col in blocks:
            j0 = blk * 4
            for b in range(BLOC):
                units.append(dict(
                    mq=(n2, b), mkv=(n1, b), wE=(j0 + 0, "E1"),
                    wV=(j0 + 2, "Wv1"), kind="attn1", fp8=True, prefetch=None))
                units.append(dict(
                    mq=(n1, b), mkv=(n2, b), wE=(j0 + 1, "E2"),
                    wV=(j0 + 3, "Wv2"), kind=("attn2", blk, b, col),
                    fp8=True, prefetch=None))
            if blk < 2:
                nj = (blk + 1) * 4
                units[-4]["prefetch"] = [(nj + 0, "E1"), (nj + 1, "E2"),
                                         (nj + 2, "Wv1"), (nj + 3, "Wv2")]
            else:
                units[-4]["prefetch"] = [(12, "E1"), (13, "Wv1")]
        for b in range(BLOC):
            units.append(dict(
                mq=("avT", b), mkv=("txtb", b), wE=(12, "E1"), wV=(13, "Wv1"),
                kind=("cross", b), fp8=False, prefetch=None))

        # --- software pipeline -------------------------------------------
        pend_pv = None
        comb_queue = []       # [(pou1, pou2, kind, created_step)]
        prev_attn1 = None

        def emit_pv(p, half):
            probsT, v, pou, recip, sums, kind, fp8 = p
            cross_b = kind[1] if (kind != "attn1" and kind[0] == "cross") else None
            qts = range(0, 4) if half == 0 else range(4, LT)
            pv_range(probsT, v, pou, recip, sums, qts, fp8, cross_b)

        def finish_pv(p, step):
            nonlocal prev_attn1
            pou_p, kind_p = p[2], p[5]
            if kind_p == "attn1":
                prev_attn1 = pou_p
            elif kind_p[0] == "attn2":
                comb_queue.append((prev_attn1, pou_p, kind_p, step))

        for step, u in enumerate(units):
            if u["prefetch"]:
                for j, tag in u["prefetch"]:
                    use_w(j, tag)
            E = use_w(*u["wE"])
            Wv = use_w(*u["wV"])
            if u["mq"][0] == "avT":
                mq = avT[u["mq"][1]]
            else:
                mq = mtT[u["mq"]]
            mkv = txtb[u["mkv"][1]] if u["mkv"][0] == "txtb" else mtT[u["mkv"]]
            tT = proj_T(E, mq, u["fp8"])
            v = proj_N(Wv, mkv, u["fp8"])
            if pend_pv is not None:
                emit_pv(pend_pv, 0)
            probsT = scores_exp(mkv, tT, u["fp8"])
            if pend_pv is not None:
                emit_pv(pend_pv, 1)
                finish_pv(pend_pv, step)
            while comb_queue and comb_queue[0][3] < step:
                pou1, pou2, kind, _ = comb_queue.pop(0)
                combine_pair(pou1, pou2, kind[1], kind[2], kind[3])
            pou = work.tile([P, LT, A], BF16, tag="pou", name="pou", bufs=3)
            recip = work.tile([P, LT], F32, tag="recip", name="recip", bufs=3)
            sums = psC.tile([P, LT], F32, tag="sums", name="sums")
            pend_pv = (probsT, v, pou, recip, sums, u["kind"], u["fp8"])
        # tail flush
        emit_pv(pend_pv, 0)
        emit_pv(pend_pv, 1)
        finish_pv(pend_pv, len(units))
        for pou1, pou2, kind, _ in comb_queue:
            combine_pair(pou1, pou2, kind[1], kind[2], kind[3])


_nc_cache = None
last_results = None


def _get_nc():
    global _nc_cache
    if _nc_cache is None:
        _nc_cache = _build()
    return _nc_cache


def kernel(**inputs):
    global last_results
    txt = np.asarray(inputs["txt"], dtype=np.float32)
    au = np.asarray(inputs["au"], dtype=np.float32)
    vi = np.asarray(inputs["vi"], dtype=np.float32)

    nat = {"txt": txt, "au": au, "vi": vi}
    mtn = {n: np.ascontiguousarray(v.transpose(0, 2, 1)) for n, v in nat.items()}
    mt8 = {n: v.astype(ml_dtypes.float8_e4m3) for n, v in mtn.items()}
    txt_bf = mtn["txt"].astype(ml_dtypes.bfloat16)

    g = {n: np.asarray(inputs[n], dtype=np.float32) for n in inputs}
    # sym weights: E = WSC * Wq^T @ Wk ; WvT = WSC * Wv^T  -> fp8
    wlist = []
    for blk in ("ta", "va", "tv"):
        wlist += [
            WSC * (g[f"{blk}_qy"].T @ g[f"{blk}_kx"]),
            WSC * (g[f"{blk}_qx"].T @ g[f"{blk}_ky"]),
            WSC * g[f"{blk}_vx"].T,
            WSC * g[f"{blk}_vy"].T,
        ]
    wt_all = np.ascontiguousarray(np.stack(wlist)).astype(ml_dtypes.float8_e4m3)
    # cross weights: unscaled, bf16
    wtb_all = np.ascontiguousarray(np.stack(
        [g["tav_q"].T @ g["tav_k"], g["tav_v"].T])).astype(ml_dtypes.bfloat16)

    res_all = np.stack([txt + au, vi + au, txt + vi])

    in_maps = []
    for c in range(NCORES):
        sl = slice(c * BLOC, (c + 1) * BLOC)
        in_maps.append({
            "mt_txt": mt8["txt"][sl],
            "mt_au": mt8["au"][sl],
            "mt_vi": mt8["vi"][sl],
            "mt_txtb": txt_bf[sl],
            "res": np.ascontiguousarray(res_all[:, sl]),
            "wt": wt_all,
            "wtb": wtb_all,
        })

    nc = _get_nc()
    last_results = run_bass_kernel_spmd(nc, in_maps, core_ids=list(range(NCORES)))
    core_out = np.concatenate(
        [np.asarray(last_results.results[c]["out"]) for c in range(NCORES)], axis=0)
    return np.concatenate([txt, au, vi, core_out], axis=-1).astype(np.float32)


# revision 17
# speedup vs baseline: 2.0453x; 1.0078x over previous
"""v6: mixed-precision E-fused attention. Six symmetric attentions in fp8
(e4m3) DoubleRow; the sharp-softmax cross attention on bf16. Transposed-score
layout (exp emits probsT directly), softmax sums via interleaved N=1 matmuls,
PV normalized in the PSUM->SBUF copy, residual combines deferred two pipeline
stages with bf16 adds (DVE 2x), bf16 outputs upcast on host. PV of the
previous unit is interleaved into the scores loop to hide exp latency.
Data-parallel: 2 batches/core, 8 cores."""

import numpy as np
import ml_dtypes

from concourse import bacc, bass, tile, mybir
from concourse.bass_utils import run_bass_kernel_spmd

B, L, D = 16, 1024, 512
A = D
NCORES = 8
BLOC = B // NCORES
P = 128
DC = D // P          # 4 contraction chunks of 128
LT = L // P          # 8 row blocks
KC = L // P          # 8 key blocks
SCALE = float(1.0 / np.sqrt(np.float32(D)))
WSC = 16.0           # host weight pre-scale for fp8 (subnormal avoidance)
EB = -2.0            # exp bias: probs scaled by e^-2, cancels in softmax

F32 = mybir.dt.float32
BF16 = mybir.dt.bfloat16
FP8 = mybir.dt.float8e4
DR = mybir.MatmulPerfMode.DoubleRow
EXP = mybir.ActivationFunctionType.Exp
COPY = mybir.ActivationFunctionType.Copy


def _build():
    nc = bacc.Bacc("TRN2", target_bir_lowering=False, debug=False,
                   num_devices=NCORES)

    mt_txt = nc.dram_tensor("mt_txt", (BLOC, D, L), FP8, kind="ExternalInput").ap()
    mt_au = nc.dram_tensor("mt_au", (BLOC, D, L), FP8, kind="ExternalInput").ap()
    mt_vi = nc.dram_tensor("mt_vi", (BLOC, D, L), FP8, kind="ExternalInput").ap()
    mt_txtb = nc.dram_tensor("mt_txtb", (BLOC, D, L), BF16,
                             kind="ExternalInput").ap()
    res = nc.dram_tensor("res", (3, BLOC, L, D), BF16, kind="ExternalInput").ap()
    wt = nc.dram_tensor("wt", (12, D, A), FP8, kind="ExternalInput").ap()
    wtb = nc.dram_tensor("wtb", (2, D, A), BF16, kind="ExternalInput").ap()
    out = nc.dram_tensor("out", (BLOC, L, 4 * A), BF16, kind="ExternalOutput").ap()

    with tile.TileContext(nc) as tc:
        _body(nc, tc, mt_txt, mt_au, mt_vi, mt_txtb, res, wt, wtb, out)

    nc.compile()
    return nc


def _body(nc, tc, mt_txt, mt_au, mt_vi, mt_txtb, res, wt, wtb, out):
    mt_dram = {"txt": mt_txt, "au": mt_au, "vi": mt_vi}

    with (
        tc.tile_pool(name="persist", bufs=1) as persist,
        tc.tile_pool(name="wpool", bufs=2) as wpool,
        tc.tile_pool(name="work", bufs=2) as work,
        tc.tile_pool(name="small", bufs=3) as smallp,
        tc.tile_pool(name="ps_score", bufs=2, space=bass.MemorySpace.PSUM) as psA,
        tc.tile_pool(name="ps_mm", bufs=3, space=bass.MemorySpace.PSUM) as psB,
        tc.tile_pool(name="ps_sums", bufs=1, space=bass.MemorySpace.PSUM) as psC,
    ):
        # --- persistent tiles; first-needed inputs first, weights on the
        # scalar DMA queue so they issue in parallel with the sync queue ------
        mtT = {}

        def load_mt(name, b):
            t = persist.tile([P, DC, L], FP8, tag=f"mt_{name}{b}",
                             name=f"mt_{name}{b}")
            nc.sync.dma_start(
                out=t[:, :, :],
                in_=mt_dram[name][b].rearrange("(dc p) l -> p dc l", p=P))
            mtT[(name, b)] = t

        wtiles = {}

        def use_w(j, tag):
            # j >= 12 -> bf16 cross weights from wtb
            if j not in wtiles:
                if j >= 12:
                    t = wpool.tile([P, DC, A], BF16, tag=tag, name=f"w{j}")
                    nc.scalar.dma_start(
                        out=t[:, :, :],
                        in_=wtb[j - 12].rearrange("(dc p) a -> p dc a", p=P))
                else:
                    t = wpool.tile([P, DC, A], FP8, tag=tag, name=f"w{j}")
                    nc.scalar.dma_start(
                        out=t[:, :, :],
                        in_=wt[j].rearrange("(dc p) a -> p dc a", p=P))
                wtiles[j] = t
            return wtiles[j]

        for j, tag in ((0, "E1"), (2, "Wv1"), (1, "E2"), (3, "Wv2")):
            use_w(j, tag)
        load_mt("txt", 0)
        load_mt("au", 0)
        load_mt("vi", 0)
        load_mt("txt", 1)
        load_mt("au", 1)
        load_mt("vi", 1)
        txtb = []
        for b in range(BLOC):
            t = persist.tile([P, DC, L], BF16, tag=f"txtb{b}", name=f"txtb{b}")
            nc.sync.dma_start(
                out=t[:, :, :],
                in_=mt_txtb[b].rearrange("(dc p) l -> p dc l", p=P))
            txtb.append(t)

        avT = [persist.tile([P, DC, L], BF16, tag=f"avT{b}", name=f"avT{b}")
               for b in range(BLOC)]
        onesDR = persist.tile([P, 2, 1], FP8, tag="onesDR", name="onesDR")
        nc.vector.memset(onesDR[:, :, :], WSC)
        ones_bf = persist.tile([P, 1], BF16, tag="ones_bf", name="ones_bf")
        nc.vector.memset(ones_bf[:, :], 1.0)
        ebias = persist.tile([P, 1], F32, tag="ebias", name="ebias")
        nc.vector.memset(ebias[:, :], EB)

        # --- per-unit phases --------------------------------------------------
        def proj_T(E, mqT, fp8):
            tT = work.tile([P, DC, L], FP8 if fp8 else BF16, tag="tT", name="tT")
            for db in range(DC):
                for qh in range(2):
                    ps = psB.tile([P, 512], F32, tag="mm", name="ps_pt")
                    if fp8:
                        for dcp in (0, 2):
                            nc.tensor.matmul(
                                ps[:, :],
                                E[:, dcp:dcp + 2, db * P:(db + 1) * P],
                                mqT[:, dcp:dcp + 2, qh * 512:(qh + 1) * 512],
                                start=(dcp == 0), stop=(dcp == 2), perf_mode=DR)
                    else:
                        for dc in range(DC):
                            nc.tensor.matmul(
                                ps[:, :],
                                E[:, dc, db * P:(db + 1) * P],
                                mqT[:, dc, qh * 512:(qh + 1) * 512],
                                start=(dc == 0), stop=(dc == DC - 1))
                    nc.vector.tensor_copy(tT[:, db, qh * 512:(qh + 1) * 512],
                                          ps[:, :])
            return tT

        def proj_N(WvT, mkvT, fp8):
            v = work.tile([P, KC, A], FP8 if fp8 else BF16, tag="v", name="v")
            for lt in range(LT):
                ps = psB.tile([P, A], F32, tag="mm", name="ps_pn")
                if fp8:
                    for dcp in (0, 2):
                        nc.tensor.matmul(ps[:, :],
                                         mkvT[:, dcp:dcp + 2, lt * P:(lt + 1) * P],
                                         WvT[:, dcp:dcp + 2, :],
                                         start=(dcp == 0), stop=(dcp == 2),
                                         perf_mode=DR)
                else:
                    for dc in range(DC):
                        nc.tensor.matmul(ps[:, :],
                                         mkvT[:, dc, lt * P:(lt + 1) * P],
                                         WvT[:, dc, :],
                                         start=(dc == 0), stop=(dc == DC - 1))
                nc.scalar.activation(v[:, lt, :], ps[:, :], COPY)
            return v

        def scores_kt(mkvT, tT, probsT, kt, fp8):
            ps = psA.tile([P, L], F32, tag="score", name="ps_sc")
            for qh in range(2):
                if fp8:
                    for dcp in (0, 2):
                        nc.tensor.matmul(
                            ps[:, qh * 512:(qh + 1) * 512],
                            mkvT[:, dcp:dcp + 2, kt * P:(kt + 1) * P],
                            tT[:, dcp:dcp + 2, qh * 512:(qh + 1) * 512],
                            start=(dcp == 0), stop=(dcp == 2), perf_mode=DR)
                else:
                    for dc in range(DC):
                        nc.tensor.matmul(
                            ps[:, qh * 512:(qh + 1) * 512],
                            mkvT[:, dc, kt * P:(kt + 1) * P],
                            tT[:, dc, qh * 512:(qh + 1) * 512],
                            start=(dc == 0), stop=(dc == DC - 1))
            nc.scalar.activation(probsT[:, kt, :], ps[:, :], EXP,
                                 scale=(SCALE / WSC if fp8 else SCALE),
                                 bias=ebias[:, :])

        def pv_qt(p, qt):
            """One PV qt-group of a pending unit p (normalized into pou)."""
            probsT, v, pou, recip, sums, kind, fp8 = p
            cross_b = kind[1] if (kind != "attn1" and kind[0] == "cross") else None
            po = psB.tile([P, A], F32, tag="mm", name="ps_pv")
            if fp8:
                for kcp in (0, 2, 4, 6):
                    w = probsT[:, kcp:kcp + 2, qt * P:(qt + 1) * P]
                    nc.tensor.matmul(po[:, :], w, v[:, kcp:kcp + 2, :],
                                     start=(kcp == 0), stop=(kcp == 6),
                                     perf_mode=DR)
                    nc.tensor.matmul(sums[:, qt:qt + 1], w, onesDR[:, :, :],
                                     start=(kcp == 0), stop=(kcp == 6),
                                     perf_mode=DR)
            else:
                for kc in range(KC):
                    w = probsT[:, kc, qt * P:(qt + 1) * P]
                    nc.tensor.matmul(po[:, :], w, v[:, kc, :],
                                     start=(kc == 0), stop=(kc == KC - 1))
                    nc.tensor.matmul(sums[:, qt:qt + 1], w, ones_bf[:, :],
                                     start=(kc == 0), stop=(kc == KC - 1))
            nc.vector.reciprocal(recip[:, qt:qt + 1], sums[:, qt:qt + 1])
            if cross_b is None:
                nc.vector.tensor_scalar_mul(pou[:, qt, :], po[:, :],
                                            recip[:, qt:qt + 1])
            else:
                out_c = smallp.tile([P, A], BF16, tag="out_c", name="out_c")
                nc.vector.tensor_scalar_mul(out_c[:, :], po[:, :],
                                            recip[:, qt:qt + 1])
                nc.sync.dma_start(
                    out=out[cross_b, qt * P:(qt + 1) * P, 3 * A:4 * A],
                    in_=out_c[:, :])

        # --- deferred combine: out = pou1 + pou2 + res (all bf16, DVE 2x) ----
        def combine_pair(pou1, pou2, blk, b, col):
            for qt in range(LT):
                res_t = smallp.tile([P, A], BF16, tag="res_t", name="res_t")
                nc.sync.dma_start(out=res_t[:, :],
                                  in_=res[blk, b, qt * P:(qt + 1) * P, :])
                osum = smallp.tile([P, A], BF16, tag="osum", name="osum")
                nc.vector.tensor_add(osum[:, :], pou1[:, qt, :], pou2[:, qt, :])
                nc.vector.tensor_add(osum[:, :], osum[:, :], res_t[:, :])
                nc.sync.dma_start(
                    out=out[b, qt * P:(qt + 1) * P, col * A:(col + 1) * A],
                    in_=osum[:, :])
                if blk == 1:
                    nc.sync.dma_start_transpose(
                        out=avT[b][:, :, qt * P:(qt + 1) * P],
                        in_=osum[:, :])

        # --- unit schedule ----------------------------------------------------
        # order: blk0 (4 units), blk1 (4), blk2-b0 (2), cross-b0, cross-b1,
        # blk2-b1 (2)  -- crosses mid-stream, sym tail.
        blocks = [(0, "txt", "au", 0), (1, "vi", "au", 2), (2, "txt", "vi", 1)]

        def sym_units(blk, n1, n2, col, b):
            j0 = blk * 4
            return [
                dict(mq=(n2, b), mkv=(n1, b), wE=(j0 + 0, "E1"),
                     wV=(j0 + 2, "Wv1"), kind="attn1", fp8=True, prefetch=None),
                dict(mq=(n1, b), mkv=(n2, b), wE=(j0 + 1, "E2"),
                     wV=(j0 + 3, "Wv2"), kind=("attn2", blk, b, col),
                     fp8=True, prefetch=None),
            ]

        def cross_unit(b):
            return dict(mq=("avT", b), mkv=("txtb", b), wE=(12, "E1"),
                        wV=(13, "Wv1"), kind=("cross", b), fp8=False,
                        prefetch=None)

        units = []
        for b in range(BLOC):
            units += sym_units(0, "txt", "au", 0, b)
        for b in range(BLOC):
            units += sym_units(1, "vi", "au", 2, b)
        units += sym_units(2, "txt", "vi", 1, 0)
        units += [cross_unit(0), cross_unit(1)]
        units += sym_units(2, "txt", "vi", 1, 1)
        # weight prefetches: next block's weights at the previous block's start
        units[0]["prefetch"] = [(4, "E1"), (5, "E2"), (6, "Wv1"), (7, "Wv2")]
        units[4]["prefetch"] = [(8, "E1"), (9, "E2"), (10, "Wv1"), (11, "Wv2")]
        units[8]["prefetch"] = [(12, "E1"), (13, "Wv1")]

        # --- software pipeline -------------------------------------------
        # Unit step i: proj(i), pvA(i-1), scores(i) with pvB(i-1) interleaved,
        # then combines created at step <= i-1.
        pend_pv = None
        comb_queue = []       # [(pou1, pou2, kind, created_step)]
        prev_attn1 = None

        def finish_pv(p, step):
            nonlocal prev_attn1
            pou_p, kind_p = p[2], p[5]
            if kind_p == "attn1":
                prev_attn1 = pou_p
            elif kind_p[0] == "attn2":
                comb_queue.append((prev_attn1, pou_p, kind_p, step))

        for step, u in enumerate(units):
            if u["prefetch"]:
                for j, tag in u["prefetch"]:
                    use_w(j, tag)
            E = use_w(*u["wE"])
            Wv = use_w(*u["wV"])
            mq = avT[u["mq"][1]] if u["mq"][0] == "avT" else mtT[u["mq"]]
            mkv = txtb[u["mkv"][1]] if u["mkv"][0] == "txtb" else mtT[u["mkv"]]
            tT = proj_T(E, mq, u["fp8"])
            v = proj_N(Wv, mkv, u["fp8"])
            if pend_pv is not None:
                for qt in range(4):
                    pv_qt(pend_pv, qt)
            probsT = work.tile([P, KC, L], FP8 if u["fp8"] else BF16,
                               tag="probsT", name="probsT")
            # scores with pvB of the previous unit interleaved (hides exp)
            for kt in range(KC):
                scores_kt(mkv, tT, probsT, kt, u["fp8"])
                if pend_pv is not None and kt in (1, 3, 5, 7):
                    pv_qt(pend_pv, 4 + (kt - 1) // 2)
            if pend_pv is not None:
                finish_pv(pend_pv, step)
            while comb_queue and comb_queue[0][3] < step:
                pou1, pou2, kind, _ = comb_queue.pop(0)
                combine_pair(pou1, pou2, kind[1], kind[2], kind[3])
            pou = work.tile([P, LT, A], BF16, tag="pou", name="pou", bufs=3)
            recip = work.tile([P, LT], F32, tag="recip", name="recip", bufs=3)
            sums = psC.tile([P, LT], F32, tag="sums", name="sums")
            pend_pv = (probsT, v, pou, recip, sums, u["kind"], u["fp8"])
        # tail flush
        for qt in range(LT):
            pv_qt(pend_pv, qt)
        finish_pv(pend_pv, len(units))
        for pou1, pou2, kind, _ in comb_queue:
            combine_pair(pou1, pou2, kind[1], kind[2], kind[3])


_nc_cache = None
last_results = None


def _get_nc():
    global _nc_cache
    if _nc_cache is None:
        _nc_cache = _build()
    return _nc_cache


def kernel(**inputs):
    global last_results
    txt = np.asarray(inputs["txt"], dtype=np.float32)
    au = np.asarray(inputs["au"], dtype=np.float32)
    vi = np.asarray(inputs["vi"], dtype=np.float32)

    nat = {"txt": txt, "au": au, "vi": vi}
    mtn = {n: np.ascontiguousarray(v.transpose(0, 2, 1)) for n, v in nat.items()}
    mt8 = {n: v.astype(ml_dtypes.float8_e4m3) for n, v in mtn.items()}
    txt_bf = mtn["txt"].astype(ml_dtypes.bfloat16)

    g = {n: np.asarray(inputs[n], dtype=np.float32) for n in inputs}
    wlist = []
    for blk in ("ta", "va", "tv"):
        wlist += [
            WSC * (g[f"{blk}_qy"].T @ g[f"{blk}_kx"]),
            WSC * (g[f"{blk}_qx"].T @ g[f"{blk}_ky"]),
            WSC * g[f"{blk}_vx"].T,
            WSC * g[f"{blk}_vy"].T,
        ]
    wt_all = np.ascontiguousarray(np.stack(wlist)).astype(ml_dtypes.float8_e4m3)
    wtb_all = np.ascontiguousarray(np.stack(
        [g["tav_q"].T @ g["tav_k"], g["tav_v"].T])).astype(ml_dtypes.bfloat16)

    res_all = np.stack([txt + au, vi + au, txt + vi]).astype(ml_dtypes.bfloat16)

    in_maps = []
    for c in range(NCORES):
        sl = slice(c * BLOC, (c + 1) * BLOC)
        in_maps.append({
            "mt_txt": mt8["txt"][sl],
            "mt_au": mt8["au"][sl],
            "mt_vi": mt8["vi"][sl],
            "mt_txtb": txt_bf[sl],
            "res": np.ascontiguousarray(res_all[:, sl]),
            "wt": wt_all,
            "wtb": wtb_all,
        })

    nc = _get_nc()
    last_results = run_bass_kernel_spmd(nc, in_maps, core_ids=list(range(NCORES)))
    core_out = np.concatenate(
        [np.asarray(last_results.results[c]["out"]).astype(np.float32)
         for c in range(NCORES)], axis=0)
    return np.concatenate([txt, au, vi, core_out], axis=-1).astype(np.float32)
